# revision 13
# baseline (speedup 1.0000x reference)
"""Trainium2 Bass kernel for nn_Conv2DLayer_16011638080159.

Math: out = C * (x @ weight.sum(0))   with x [524288, 512], weight [9, 512].
Equivalent to a row-wise dot product of x with w_eff = C * weight.sum(0).

Strategy (pure data parallel, per sharding hint):
  - Shard x along the batch axis across 8 NeuronCores (65536 rows each).
  - Host-side prep: fold the tiny K=9 weight sum and the C scale into a
    single [C] vector; cast x and the folded weight to bf16 on the host
    so the device streams half the bytes (~67 MB/core) and DVE's 2x bf16
    mode applies. fp32 accumulation keeps l2 error ~3e-3, inside the
    2e-2 gate.
  - Per core: stream x in [128, 8192] bf16 tiles on the SP HWDGE queue
    ONLY (a single queue streams at ~400 GB/s; splitting across two
    HWDGE queues measured ~25% slower). The tiny [128, 512] weight rides
    the ACT queue once and is replicated to [128, 8192] on device.
  - Compute is the bottleneck (~7.4 us/tile across two engines):
      * DVE: bf16 tile-wide multiply (2x), then for the first S_DVE row
        slots a two-level pairwise-halving tree of bf16 adds (also 2x)
        followed by one segmented add-reduce (fp32 accum) on the
        128-wide remainders.
      * ACT: per-row ACTIVATE(Copy, accum_out) for the other rows.
  - Row mapping: shard row (p*512 + t*R + r) sits at partition p, tile t,
    slot r, so the per-core result tile [128, 512] is exactly the row-major
    view of the per-core output [65536]; one contiguous DMA writes it out.
"""

import numpy as np
import ml_dtypes

import concourse.bacc as bacc
import concourse.bass as bass
import concourse.tile as tile
from concourse import mybir
from concourse.bass_utils import run_bass_kernel_spmd

B = 524288        # total rows
C = 512           # row length
N_CORES = 8
BS = B // N_CORES  # 65536 rows per core
P = 128            # SBUF partitions
RPP = BS // P      # 512 rows per partition
R = 16             # rows per partition per tile
F = R * C          # free elems per tile (2 MB bf16)
NT = RPP // R      # 32 tiles per core
S_DVE = 8          # row slots reduced on DVE (halving tree); rest on ACT
KGP = 3            # every KGP-th tile is multiplied on GpSimd (own buffers)
H1 = C // 2        # 256
H2 = C // 4        # 128

_NC_CACHE = None
LAST_RESULT = None  # BassKernelResults of the most recent run (for profiling)


def _build() -> bass.Bass:
    # Bacc (not raw Bass): its compile() pass splits multi-sem waits into
    # EventSemaphore instructions -- the TRN2 ISA allows only 1 wait/inst.
    nc = bacc.Bacc(None, target_bir_lowering=False, debug=False)
    x = nc.dram_tensor("x", [BS, C], mybir.dt.bfloat16, kind="ExternalInput")
    w = nc.dram_tensor("w", [P, C], mybir.dt.bfloat16, kind="ExternalInput")
    out = nc.dram_tensor("out", [BS], mybir.dt.float32, kind="ExternalOutput")

    # shard row (p*RPP + t*R + r) -> partition p, tile t, free slot (r, c)
    xv = x.rearrange("(p t r) c -> t p (r c)", p=P, t=NT, r=R)
    ov = out.rearrange("(p f) -> p f", p=P)

    n_act = R - S_DVE

    with tile.TileContext(nc) as tc:
        with (
            tc.tile_pool(name="const", bufs=1) as cpool,
            tc.tile_pool(name="xs", bufs=4) as xs,
            tc.tile_pool(name="ys", bufs=3) as ys,
            tc.tile_pool(name="yg", bufs=2) as ygp,
            tc.tile_pool(name="h1", bufs=2) as h1p,
            tc.tile_pool(name="h2", bufs=2) as h2p,
            tc.tile_pool(name="scr", bufs=2) as scr,
            tc.tile_pool(name="res", bufs=1) as res,
        ):
            # tiny w first in the SP HWDGE FIFO (~0.4 us ahead of x tile 0;
            # on the ACT queue it interleaves with the x stream and takes
            # ~10 us); doubling-replicate on DVE overlaps x tile 0's DMA
            w_t = cpool.tile([P, C], mybir.dt.bfloat16)
            nc.sync.dma_start(out=w_t[:], in_=w[:, :])
            wb_t = cpool.tile([P, F], mybir.dt.bfloat16)
            nc.vector.tensor_copy(out=wb_t[:, 0:C], in_=w_t[:])
            rep = C
            while rep < F:
                n = min(rep, F - rep)
                nc.vector.tensor_copy(
                    out=wb_t[:, rep:rep + n], in_=wb_t[:, 0:n])
                rep += n
            # GpSimd works from its own w copy and its own y tiles so the
            # engines never read/write the same SBUF buffers concurrently
            # (same-tile co-runs measurably stretch DVE ~1.5x)
            wg_t = cpool.tile([P, F], mybir.dt.bfloat16)
            nc.vector.tensor_copy(out=wg_t[:], in_=wb_t[:])
            o_t = res.tile([P, RPP], mybir.dt.float32)

            for t in range(NT):
                x_t = xs.tile([P, F], mybir.dt.bfloat16)
                nc.sync.dma_start(out=x_t[:], in_=xv[t])

                # bf16 multiply: every KGP-th tile on GpSimd (into its own
                # tile, from its own w copy), the rest on DVE in 2x mode
                if t % KGP == KGP - 1:
                    y_t = ygp.tile([P, F], mybir.dt.bfloat16)
                    nc.gpsimd.tensor_tensor(
                        y_t[:], x_t[:], wg_t[:], op=mybir.AluOpType.mult)
                else:
                    y_t = ys.tile([P, F], mybir.dt.bfloat16)
                    nc.vector.tensor_mul(y_t[:], x_t[:], wb_t[:])
                y3 = y_t[:, 0:S_DVE * C].rearrange("p (r c) -> p r c", c=C)

                # DVE: two halving levels (2x) for the first S_DVE rows
                h1_t = h1p.tile([P, S_DVE * H1], mybir.dt.bfloat16)
                h1v = h1_t[:].rearrange("p (r c) -> p r c", c=H1)
                nc.vector.tensor_add(h1v, y3[:, :, 0:H1], y3[:, :, H1:C])
                h2_t = h2p.tile([P, S_DVE * H2], mybir.dt.bfloat16)
                h2v = h2_t[:].rearrange("p (r c) -> p r c", c=H2)
                nc.vector.tensor_add(h2v, h1v[:, :, 0:H2], h1v[:, :, H2:H1])

                # DVE: segmented add-reduce of the 128-wide remainders
                nc.vector.tensor_reduce(
                    out=o_t[:, t * R: t * R + S_DVE],
                    in_=h2v,
                    axis=mybir.AxisListType.X,
                    op=mybir.AluOpType.add,
                )

                # ACT: accumulate the remaining rows (one 512-sum per row)
                for r in range(n_act):
                    s_t = scr.tile([P, C], mybir.dt.bfloat16, tag="act_s")
                    col = t * R + S_DVE + r
                    nc.scalar.activation(
                        out=s_t[:],
                        in_=y_t[:, (S_DVE + r) * C:(S_DVE + r + 1) * C],
                        func=mybir.ActivationFunctionType.Copy,
                        accum_out=o_t[:, col: col + 1],
                    )
            nc.sync.dma_start(out=ov, in_=o_t[:])
    nc.finalize()
    return nc


def kernel(x: np.ndarray, weight: np.ndarray) -> np.ndarray:
    global _NC_CACHE, LAST_RESULT
    x = np.asarray(x)
    weight = np.asarray(weight, dtype=np.float32)

    x16 = np.ascontiguousarray(x.astype(ml_dtypes.bfloat16))
    w_eff = (C * weight.sum(axis=0)).astype(ml_dtypes.bfloat16)  # [C]
    w_rep = np.ascontiguousarray(np.tile(w_eff, (P, 1)))         # [P, C]

    if _NC_CACHE is None:
        _NC_CACHE = _build()

    in_maps = [
        {"x": x16[i * BS:(i + 1) * BS], "w": w_rep} for i in range(N_CORES)
    ]
    LAST_RESULT = run_bass_kernel_spmd(
        _NC_CACHE, in_maps, core_ids=list(range(N_CORES))
    )
    return np.concatenate([r["out"] for r in LAST_RESULT.results])


# revision 15
# speedup vs baseline: 1.4456x; 1.4456x over previous
"""Trainium2 Bass kernel for nn_Conv2DLayer_16011638080159.

Math: out = C * (x @ weight.sum(0))   with x [524288, 512], weight [9, 512].
Equivalent to a row-wise dot product of x with w_eff = C * weight.sum(0).

Strategy (pure data parallel, per sharding hint):
  - Shard x along the batch axis across 8 NeuronCores (65536 rows each).
  - Host-side prep: fold the tiny K=9 weight sum and the C scale into a
    single [C] vector; cast x and the folded weight to bf16 on the host
    so the device streams half the bytes (~67 MB/core) and DVE's 2x bf16
    mode applies. fp32 accumulation keeps l2 error ~3e-3, inside the
    2e-2 gate.
  - Per core: stream x in [128, 8192] bf16 tiles on the SP HWDGE queue
    ONLY (a single queue streams at ~400 GB/s; splitting across two
    HWDGE queues measured ~25% slower). The tiny [128, 512] weight rides
    the ACT queue once and is replicated to [128, 8192] on device.
  - Compute is the bottleneck (~7.4 us/tile across two engines):
      * DVE: bf16 tile-wide multiply (2x), then for the first S_DVE row
        slots a two-level pairwise-halving tree of bf16 adds (also 2x)
        followed by one segmented add-reduce (fp32 accum) on the
        128-wide remainders.
      * ACT: per-row ACTIVATE(Copy, accum_out) for the other rows.
  - Row mapping: shard row (p*512 + t*R + r) sits at partition p, tile t,
    slot r, so the per-core result tile [128, 512] is exactly the row-major
    view of the per-core output [65536]; one contiguous DMA writes it out.
"""

import numpy as np
import ml_dtypes

import concourse.bacc as bacc
import concourse.bass as bass
import concourse.tile as tile
from concourse import mybir
from concourse.bass_utils import run_bass_kernel_spmd

B = 524288        # total rows
C = 512           # row length
N_CORES = 8
BS = B // N_CORES  # 65536 rows per core
P = 128            # SBUF partitions
RPP = BS // P      # 512 rows per partition
R = 16             # rows per partition per tile
F = R * C          # free elems per tile (2 MB bf16)
NT = RPP // R      # 32 tiles per core
S_DVE = 7          # row slots reduced on DVE (halving tree); rest on ACT
RT = 4             # rows per head chunk: tile 0 is split into 4 quarter
FT = RT * C        # chunks so the first multiply starts ~5 us earlier
H1 = C // 2        # 256
H2 = C // 4        # 128

_NC_CACHE = None
LAST_RESULT = None  # BassKernelResults of the most recent run (for profiling)


def _build() -> bass.Bass:
    # Bacc (not raw Bass): its compile() pass splits multi-sem waits into
    # EventSemaphore instructions -- the TRN2 ISA allows only 1 wait/inst.
    nc = bacc.Bacc(None, target_bir_lowering=False, debug=False)
    x = nc.dram_tensor("x", [BS, C], mybir.dt.bfloat16, kind="ExternalInput")
    w = nc.dram_tensor("w", [P, C], mybir.dt.bfloat16, kind="ExternalInput")
    out = nc.dram_tensor("out", [BS], mybir.dt.float32, kind="ExternalOutput")

    # shard row (p*RPP + t*R + r) -> partition p, tile t, free slot (r, c)
    xv = x.rearrange("(p t r) c -> t p (r c)", p=P, t=NT, r=R)
    # same rows at quarter-tile granularity for the head chunks
    xw = x.rearrange("(p s r) c -> s p (r c)", p=P, s=RPP // RT, r=RT)
    ov = out.rearrange("(p f) -> p f", p=P)

    n_act = R - S_DVE

    with tile.TileContext(nc) as tc:
        with (
            tc.tile_pool(name="const", bufs=1) as cpool,
            tc.tile_pool(name="xs", bufs=5) as xs,
            tc.tile_pool(name="ys", bufs=3) as ys,
            tc.tile_pool(name="h1", bufs=2) as h1p,
            tc.tile_pool(name="h2", bufs=2) as h2p,
            tc.tile_pool(name="scr", bufs=2) as scr,
            tc.tile_pool(name="res", bufs=1) as res,
        ):
            # tiny w first in the SP HWDGE FIFO (~0.4 us ahead of x tile 0;
            # on the ACT queue it interleaves with the x stream and takes
            # ~10 us); doubling-replicate on DVE overlaps x tile 0's DMA
            w_t = cpool.tile([P, C], mybir.dt.bfloat16)
            nc.sync.dma_start(out=w_t[:], in_=w[:, :])
            wb_t = cpool.tile([P, F], mybir.dt.bfloat16)
            nc.vector.tensor_copy(out=wb_t[:, 0:C], in_=w_t[:])
            rep = C
            while rep < F:
                n = min(rep, F - rep)
                nc.vector.tensor_copy(
                    out=wb_t[:, rep:rep + n], in_=wb_t[:, 0:n])
                rep += n
            o_t = res.tile([P, RPP], mybir.dt.float32)

            # head: tile 0 as 4 quarter chunks - the first chunk lands
            # ~5 us before a full 2 MB tile would, so both compute
            # engines spin up earlier; DVE seg-reduces 2 rows per chunk
            # (no tree - chunk overhead dominates), ACT takes the other 2
            for si in range(R // RT):
                xq_t = xs.tile([P, F], mybir.dt.bfloat16, tag="x")
                nc.sync.dma_start(out=xq_t[:, 0:FT], in_=xw[si])
                yq_t = ys.tile([P, F], mybir.dt.bfloat16, tag="y")
                nc.vector.tensor_mul(
                    yq_t[:, 0:FT], xq_t[:, 0:FT], wb_t[:, 0:FT])
                nc.vector.tensor_reduce(
                    out=o_t[:, si * RT: si * RT + 2],
                    in_=yq_t[:, 0:2 * C].rearrange("p (r c) -> p r c", c=C),
                    axis=mybir.AxisListType.X,
                    op=mybir.AluOpType.add,
                )
                for r in range(2, RT):
                    s_t = scr.tile([P, C], mybir.dt.bfloat16, tag="act_s")
                    col = si * RT + r
                    nc.scalar.activation(
                        out=s_t[:],
                        in_=yq_t[:, r * C:(r + 1) * C],
                        func=mybir.ActivationFunctionType.Copy,
                        accum_out=o_t[:, col: col + 1],
                    )

            for t in range(1, NT):
                x_t = xs.tile([P, F], mybir.dt.bfloat16, tag="x")
                nc.sync.dma_start(out=x_t[:], in_=xv[t])

                # DVE: bf16 multiply, 2x mode
                y_t = ys.tile([P, F], mybir.dt.bfloat16, tag="y")
                nc.vector.tensor_mul(y_t[:], x_t[:], wb_t[:])
                y3 = y_t[:, 0:S_DVE * C].rearrange("p (r c) -> p r c", c=C)

                # DVE: two halving levels (2x) for the first S_DVE rows
                h1_t = h1p.tile([P, S_DVE * H1], mybir.dt.bfloat16)
                h1v = h1_t[:].rearrange("p (r c) -> p r c", c=H1)
                nc.vector.tensor_add(h1v, y3[:, :, 0:H1], y3[:, :, H1:C])
                h2_t = h2p.tile([P, S_DVE * H2], mybir.dt.bfloat16)
                h2v = h2_t[:].rearrange("p (r c) -> p r c", c=H2)
                nc.vector.tensor_add(h2v, h1v[:, :, 0:H2], h1v[:, :, H2:H1])

                # DVE: segmented add-reduce of the 128-wide remainders
                nc.vector.tensor_reduce(
                    out=o_t[:, t * R: t * R + S_DVE],
                    in_=h2v,
                    axis=mybir.AxisListType.X,
                    op=mybir.AluOpType.add,
                )

                # ACT: accumulate the remaining rows (one 512-sum per row)
                for r in range(n_act):
                    s_t = scr.tile([P, C], mybir.dt.bfloat16, tag="act_s")
                    col = t * R + S_DVE + r
                    nc.scalar.activation(
                        out=s_t[:],
                        in_=y_t[:, (S_DVE + r) * C:(S_DVE + r + 1) * C],
                        func=mybir.ActivationFunctionType.Copy,
                        accum_out=o_t[:, col: col + 1],
                    )
            nc.sync.dma_start(out=ov, in_=o_t[:])
    nc.finalize()
    return nc


def kernel(x: np.ndarray, weight: np.ndarray) -> np.ndarray:
    global _NC_CACHE, LAST_RESULT
    x = np.asarray(x)
    weight = np.asarray(weight, dtype=np.float32)

    x16 = np.ascontiguousarray(x.astype(ml_dtypes.bfloat16))
    w_eff = (C * weight.sum(axis=0)).astype(ml_dtypes.bfloat16)  # [C]
    w_rep = np.ascontiguousarray(np.tile(w_eff, (P, 1)))         # [P, C]

    if _NC_CACHE is None:
        _NC_CACHE = _build()

    in_maps = [
        {"x": x16[i * BS:(i + 1) * BS], "w": w_rep} for i in range(N_CORES)
    ]
    LAST_RESULT = run_bass_kernel_spmd(
        _NC_CACHE, in_maps, core_ids=list(range(N_CORES))
    )
    return np.concatenate([r["out"] for r in LAST_RESULT.results])


# revision 16
# speedup vs baseline: 1.4482x; 1.0018x over previous
"""Trainium2 Bass kernel for nn_Conv2DLayer_16011638080159.

Math: out = C * (x @ weight.sum(0))   with x [524288, 512], weight [9, 512].
Equivalent to a row-wise dot product of x with w_eff = C * weight.sum(0).

Strategy (pure data parallel, per sharding hint):
  - Shard x along the batch axis across 8 NeuronCores (65536 rows each).
  - Host-side prep: fold the tiny K=9 weight sum and the C scale into a
    single [C] vector; cast x and the folded weight to bf16 on the host
    so the device streams half the bytes (~67 MB/core) and DVE's 2x bf16
    mode applies. fp32 accumulation keeps l2 error ~3e-3, inside the
    2e-2 gate.
  - Per core: stream x in [128, 8192] bf16 tiles on the SP HWDGE queue
    ONLY (a single queue streams at ~400 GB/s; splitting across two
    HWDGE queues measured ~25% slower). The tiny [128, 512] weight rides
    the ACT queue once and is replicated to [128, 8192] on device.
  - Compute is the bottleneck (~7.4 us/tile across two engines):
      * DVE: bf16 tile-wide multiply (2x), then for the first S_DVE row
        slots a two-level pairwise-halving tree of bf16 adds (also 2x)
        followed by one segmented add-reduce (fp32 accum) on the
        128-wide remainders.
      * ACT: per-row ACTIVATE(Copy, accum_out) for the other rows.
  - Row mapping: shard row (p*512 + t*R + r) sits at partition p, tile t,
    slot r, so the per-core result tile [128, 512] is exactly the row-major
    view of the per-core output [65536]; one contiguous DMA writes it out.
"""

import numpy as np
import ml_dtypes

import concourse.bacc as bacc
import concourse.bass as bass
import concourse.tile as tile
from concourse import mybir
from concourse.bass_utils import run_bass_kernel_spmd

B = 524288        # total rows
C = 512           # row length
N_CORES = 8
BS = B // N_CORES  # 65536 rows per core
P = 128            # SBUF partitions
RPP = BS // P      # 512 rows per partition
R = 16             # rows per partition per tile
F = R * C          # free elems per tile (2 MB bf16)
NT = RPP // R      # 32 tiles per core
S_DVE = 7          # row slots reduced on DVE (halving tree); rest on ACT
H1 = C // 2        # 256
H2 = C // 4        # 128

_NC_CACHE = None
LAST_RESULT = None  # BassKernelResults of the most recent run (for profiling)


def _build() -> bass.Bass:
    # Bacc (not raw Bass): its compile() pass splits multi-sem waits into
    # EventSemaphore instructions -- the TRN2 ISA allows only 1 wait/inst.
    nc = bacc.Bacc(None, target_bir_lowering=False, debug=False)
    x = nc.dram_tensor("x", [BS, C], mybir.dt.bfloat16, kind="ExternalInput")
    w = nc.dram_tensor("w", [P, C], mybir.dt.bfloat16, kind="ExternalInput")
    out = nc.dram_tensor("out", [BS], mybir.dt.float32, kind="ExternalOutput")

    # shard row (p*RPP + t*R + r) -> partition p, tile t, free slot (r, c)
    xv = x.rearrange("(p t r) c -> t p (r c)", p=P, t=NT, r=R)
    ov = out.rearrange("(p f) -> p f", p=P)

    n_act = R - S_DVE

    with tile.TileContext(nc) as tc:
        with (
            tc.tile_pool(name="const", bufs=1) as cpool,
            tc.tile_pool(name="xs", bufs=5) as xs,
            tc.tile_pool(name="ys", bufs=3) as ys,
            tc.tile_pool(name="h1", bufs=2) as h1p,
            tc.tile_pool(name="h2", bufs=2) as h2p,
            tc.psum_pool(name="scr", bufs=2) as scr,
            tc.tile_pool(name="res", bufs=1) as res,
        ):
            # tiny w first in the SP HWDGE FIFO (~0.4 us ahead of x tile 0;
            # on the ACT queue it interleaves with the x stream and takes
            # ~10 us); doubling-replicate on DVE overlaps x tile 0's DMA
            w_t = cpool.tile([P, C], mybir.dt.bfloat16)
            nc.sync.dma_start(out=w_t[:], in_=w[:, :])
            wb_t = cpool.tile([P, F], mybir.dt.bfloat16)
            nc.vector.tensor_copy(out=wb_t[:, 0:C], in_=w_t[:])
            rep = C
            while rep < F:
                n = min(rep, F - rep)
                nc.vector.tensor_copy(
                    out=wb_t[:, rep:rep + n], in_=wb_t[:, 0:n])
                rep += n
            o_t = res.tile([P, RPP], mybir.dt.float32)

            for t in range(NT):
                x_t = xs.tile([P, F], mybir.dt.bfloat16)
                nc.sync.dma_start(out=x_t[:], in_=xv[t])

                # DVE: bf16 multiply, 2x mode
                y_t = ys.tile([P, F], mybir.dt.bfloat16)
                nc.vector.tensor_mul(y_t[:], x_t[:], wb_t[:])
                y3 = y_t[:, 0:S_DVE * C].rearrange("p (r c) -> p r c", c=C)

                # DVE: two halving levels (2x) for the first S_DVE rows
                h1_t = h1p.tile([P, S_DVE * H1], mybir.dt.bfloat16)
                h1v = h1_t[:].rearrange("p (r c) -> p r c", c=H1)
                nc.vector.tensor_add(h1v, y3[:, :, 0:H1], y3[:, :, H1:C])
                h2_t = h2p.tile([P, S_DVE * H2], mybir.dt.bfloat16)
                h2v = h2_t[:].rearrange("p (r c) -> p r c", c=H2)
                nc.vector.tensor_add(h2v, h1v[:, :, 0:H2], h1v[:, :, H2:H1])

                # DVE: segmented add-reduce of the 128-wide remainders
                nc.vector.tensor_reduce(
                    out=o_t[:, t * R: t * R + S_DVE],
                    in_=h2v,
                    axis=mybir.AxisListType.X,
                    op=mybir.AluOpType.add,
                )

                # ACT: accumulate the remaining rows (one 512-sum per row)
                for r in range(n_act):
                    s_t = scr.tile([P, C], mybir.dt.float32, tag="act_s")
                    col = t * R + S_DVE + r
                    nc.scalar.activation(
                        out=s_t[:],
                        in_=y_t[:, (S_DVE + r) * C:(S_DVE + r + 1) * C],
                        func=mybir.ActivationFunctionType.Copy,
                        accum_out=o_t[:, col: col + 1],
                    )
            nc.sync.dma_start(out=ov, in_=o_t[:])
    nc.finalize()
    return nc


def kernel(x: np.ndarray, weight: np.ndarray) -> np.ndarray:
    global _NC_CACHE, LAST_RESULT
    x = np.asarray(x)
    weight = np.asarray(weight, dtype=np.float32)

    x16 = np.ascontiguousarray(x.astype(ml_dtypes.bfloat16))
    w_eff = (C * weight.sum(axis=0)).astype(ml_dtypes.bfloat16)  # [C]
    w_rep = np.ascontiguousarray(np.tile(w_eff, (P, 1)))         # [P, C]

    if _NC_CACHE is None:
        _NC_CACHE = _build()

    in_maps = [
        {"x": x16[i * BS:(i + 1) * BS], "w": w_rep} for i in range(N_CORES)
    ]
    LAST_RESULT = run_bass_kernel_spmd(
        _NC_CACHE, in_maps, core_ids=list(range(N_CORES))
    )
    return np.concatenate([r["out"] for r in LAST_RESULT.results])


# revision 18
# speedup vs baseline: 1.4538x; 1.0039x over previous
"""Trainium2 Bass kernel for nn_Conv2DLayer_16011638080159.

Math: out = C * (x @ weight.sum(0))   with x [524288, 512], weight [9, 512].
Equivalent to a row-wise dot product of x with w_eff = C * weight.sum(0).

Strategy (pure data parallel, per sharding hint):
  - Shard x along the batch axis across 8 NeuronCores (65536 rows each).
  - Host-side prep: fold the tiny K=9 weight sum and the C scale into a
    single [C] vector; cast x and the folded weight to bf16 on the host
    so the device streams half the bytes (~67 MB/core) and DVE's 2x bf16
    mode applies. fp32 accumulation keeps l2 error ~3e-3, inside the
    2e-2 gate.
  - Per core: stream x in [128, 8192] bf16 tiles on the SP HWDGE queue
    ONLY (a single queue streams at ~400 GB/s; splitting across two
    HWDGE queues measured ~25% slower). The tiny [128, 512] weight rides
    the ACT queue once and is replicated to [128, 8192] on device.
  - Compute is the bottleneck (~7.4 us/tile across two engines):
      * DVE: bf16 tile-wide multiply (2x), then for the first S_DVE row
        slots a two-level pairwise-halving tree of bf16 adds (also 2x)
        followed by one segmented add-reduce (fp32 accum) on the
        128-wide remainders.
      * ACT: per-row ACTIVATE(Copy, accum_out) for the other rows.
  - Row mapping: shard row (p*512 + t*R + r) sits at partition p, tile t,
    slot r, so the per-core result tile [128, 512] is exactly the row-major
    view of the per-core output [65536]; one contiguous DMA writes it out.
"""

import numpy as np
import ml_dtypes

import concourse.bacc as bacc
import concourse.bass as bass
import concourse.tile as tile
from concourse import mybir
from concourse.bass_utils import run_bass_kernel_spmd

B = 524288        # total rows
C = 512           # row length
N_CORES = 8
BS = B // N_CORES  # 65536 rows per core
P = 128            # SBUF partitions
RPP = BS // P      # 512 rows per partition
R = 16             # rows per partition per tile
F = R * C          # free elems per tile (2 MB bf16)
NT = RPP // R      # 32 tiles per core
S_DVE = 7          # row slots reduced on DVE (halving tree); rest on ACT
H1 = C // 2        # 256
H2 = C // 4        # 128
H3 = C // 8        # 64

_NC_CACHE = None
LAST_RESULT = None  # BassKernelResults of the most recent run (for profiling)


def _build() -> bass.Bass:
    # Bacc (not raw Bass): its compile() pass splits multi-sem waits into
    # EventSemaphore instructions -- the TRN2 ISA allows only 1 wait/inst.
    nc = bacc.Bacc(None, target_bir_lowering=False, debug=False)
    x = nc.dram_tensor("x", [BS, C], mybir.dt.bfloat16, kind="ExternalInput")
    w = nc.dram_tensor("w", [P, C], mybir.dt.bfloat16, kind="ExternalInput")
    out = nc.dram_tensor("out", [BS], mybir.dt.float32, kind="ExternalOutput")

    # shard row (p*RPP + t*R + r) -> partition p, tile t, free slot (r, c)
    xv = x.rearrange("(p t r) c -> t p (r c)", p=P, t=NT, r=R)
    ov = out.rearrange("(p f) -> p f", p=P)

    n_act = R - S_DVE

    with tile.TileContext(nc) as tc:
        with (
            tc.tile_pool(name="const", bufs=1) as cpool,
            tc.tile_pool(name="xs", bufs=5) as xs,
            tc.tile_pool(name="ys", bufs=3) as ys,
            tc.tile_pool(name="h1", bufs=2) as h1p,
            tc.tile_pool(name="h2", bufs=2) as h2p,
            tc.tile_pool(name="h3", bufs=2) as h3p,
            tc.psum_pool(name="scr", bufs=2) as scr,
            tc.tile_pool(name="res", bufs=1) as res,
        ):
            # tiny w first in the SP HWDGE FIFO (~0.4 us ahead of x tile 0;
            # on the ACT queue it interleaves with the x stream and takes
            # ~10 us); doubling-replicate on DVE overlaps x tile 0's DMA
            w_t = cpool.tile([P, C], mybir.dt.bfloat16)
            nc.sync.dma_start(out=w_t[:], in_=w[:, :])
            wb_t = cpool.tile([P, F], mybir.dt.bfloat16)
            nc.vector.tensor_copy(out=wb_t[:, 0:C], in_=w_t[:])
            rep = C
            while rep < F:
                n = min(rep, F - rep)
                nc.vector.tensor_copy(
                    out=wb_t[:, rep:rep + n], in_=wb_t[:, 0:n])
                rep += n
            o_t = res.tile([P, RPP], mybir.dt.float32)

            for t in range(NT):
                x_t = xs.tile([P, F], mybir.dt.bfloat16)
                nc.sync.dma_start(out=x_t[:], in_=xv[t])

                # DVE: bf16 multiply, 2x mode; on tile 0 produce ACT's
                # rows first so the ACT chain spins up ~2 us earlier
                y_t = ys.tile([P, F], mybir.dt.bfloat16)
                if t == 0:
                    nc.vector.tensor_mul(
                        y_t[:, S_DVE * C:], x_t[:, S_DVE * C:],
                        wb_t[:, S_DVE * C:])
                    nc.vector.tensor_mul(
                        y_t[:, 0:S_DVE * C], x_t[:, 0:S_DVE * C],
                        wb_t[:, 0:S_DVE * C])
                else:
                    nc.vector.tensor_mul(y_t[:], x_t[:], wb_t[:])
                y3 = y_t[:, 0:S_DVE * C].rearrange("p (r c) -> p r c", c=C)

                # DVE: two halving levels (2x) for the first S_DVE rows
                h1_t = h1p.tile([P, S_DVE * H1], mybir.dt.bfloat16)
                h1v = h1_t[:].rearrange("p (r c) -> p r c", c=H1)
                nc.vector.tensor_add(h1v, y3[:, :, 0:H1], y3[:, :, H1:C])
                h2_t = h2p.tile([P, S_DVE * H2], mybir.dt.bfloat16)
                h2v = h2_t[:].rearrange("p (r c) -> p r c", c=H2)
                nc.vector.tensor_add(h2v, h1v[:, :, 0:H2], h1v[:, :, H2:H1])

                h3_t = h3p.tile([P, S_DVE * H3], mybir.dt.bfloat16)
                h3v = h3_t[:].rearrange("p (r c) -> p r c", c=H3)
                nc.vector.tensor_add(h3v, h2v[:, :, 0:H3], h2v[:, :, H3:H2])

                # DVE: segmented add-reduce of the 64-wide remainders
                nc.vector.tensor_reduce(
                    out=o_t[:, t * R: t * R + S_DVE],
                    in_=h3v,
                    axis=mybir.AxisListType.X,
                    op=mybir.AluOpType.add,
                )

                # ACT: accumulate the remaining rows (one 512-sum per row)
                for r in range(n_act):
                    s_t = scr.tile([P, C], mybir.dt.float32, tag="act_s")
                    col = t * R + S_DVE + r
                    nc.scalar.activation(
                        out=s_t[:],
                        in_=y_t[:, (S_DVE + r) * C:(S_DVE + r + 1) * C],
                        func=mybir.ActivationFunctionType.Copy,
                        accum_out=o_t[:, col: col + 1],
                    )
            nc.sync.dma_start(out=ov, in_=o_t[:])
    nc.finalize()
    return nc


def kernel(x: np.ndarray, weight: np.ndarray) -> np.ndarray:
    global _NC_CACHE, LAST_RESULT
    x = np.asarray(x)
    weight = np.asarray(weight, dtype=np.float32)

    x16 = np.ascontiguousarray(x.astype(ml_dtypes.bfloat16))
    w_eff = (C * weight.sum(axis=0)).astype(ml_dtypes.bfloat16)  # [C]
    w_rep = np.ascontiguousarray(np.tile(w_eff, (P, 1)))         # [P, C]

    if _NC_CACHE is None:
        _NC_CACHE = _build()

    in_maps = [
        {"x": x16[i * BS:(i + 1) * BS], "w": w_rep} for i in range(N_CORES)
    ]
    LAST_RESULT = run_bass_kernel_spmd(
        _NC_CACHE, in_maps, core_ids=list(range(N_CORES))
    )
    return np.concatenate([r["out"] for r in LAST_RESULT.results])


# revision 20
# speedup vs baseline: 1.5038x; 1.0343x over previous
"""Trainium2 Bass kernel for nn_Conv2DLayer_16011638080159.

Math: out = C * (x @ weight.sum(0))   with x [524288, 512], weight [9, 512].
Equivalent to a row-wise dot product of x with w_eff = C * weight.sum(0).

Strategy (pure data parallel, per sharding hint):
  - Shard x along the batch axis across 8 NeuronCores (65536 rows each).
  - Host-side prep: fold the tiny K=9 weight sum and the C scale into a
    single [C] vector; cast x and the folded weight to bf16 on the host
    so the device streams half the bytes (~67 MB/core) and DVE's 2x bf16
    mode applies. fp32 accumulation keeps l2 error ~3e-3, inside the
    2e-2 gate.
  - Per core: stream x in [128, 8192] bf16 tiles on the SP HWDGE queue
    ONLY (a single queue streams at ~400 GB/s; splitting across two
    HWDGE queues measured ~25% slower). The tiny [128, 512] weight loads
    first on the same queue and is replicated to [128, 8192] on device.
  - Compute is the bottleneck (~7.2 us/tile, both engines ~100% busy):
      * DVE: bf16 tile-wide multiply (2x), then for the first S_DVE row
        slots a three-level pairwise-halving tree of bf16 adds (also 2x)
        followed by one segmented add-reduce (fp32 accum) on the 64-wide
        remainders.
      * ACT: per-row ACTIVATE(Copy, accum_out) for the other rows
        (~0.8 us/row fixed rate), scratch output in PSUM.
      * Tile 0's ACT rows are multiplied into their own tile first (tile
        deps are whole-tile, so this starts the ACT chain ~4 us earlier);
        the last two tiles shift ACT rows onto DVE, which otherwise
        idles ~12 us while ACT drains its fixed-rate queue.
  - Row mapping: shard row (p*512 + t*R + r) sits at partition p, tile t,
    slot r, so the per-core result tile [128, 512] is exactly the row-major
    view of the per-core output [65536]; one contiguous DMA writes it out.
"""

import numpy as np
import ml_dtypes

import concourse.bacc as bacc
import concourse.bass as bass
import concourse.tile as tile
from concourse import mybir
from concourse.bass_utils import run_bass_kernel_spmd

B = 524288        # total rows
C = 512           # row length
N_CORES = 8
BS = B // N_CORES  # 65536 rows per core
P = 128            # SBUF partitions
RPP = BS // P      # 512 rows per partition
R = 16             # rows per partition per tile
F = R * C          # free elems per tile (2 MB bf16)
NT = RPP // R      # 32 tiles per core
S_DVE = 7          # row slots reduced on DVE in steady state; rest on ACT
H1 = C // 2        # 256
H2 = C // 4        # 128
H3 = C // 8        # 64

_NC_CACHE = None
LAST_RESULT = None  # BassKernelResults of the most recent run (for profiling)


def _build() -> bass.Bass:
    # Bacc (not raw Bass): its compile() pass splits multi-sem waits into
    # EventSemaphore instructions -- the TRN2 ISA allows only 1 wait/inst.
    nc = bacc.Bacc(None, target_bir_lowering=False, debug=False)
    x = nc.dram_tensor("x", [BS, C], mybir.dt.bfloat16, kind="ExternalInput")
    w = nc.dram_tensor("w", [P, C], mybir.dt.bfloat16, kind="ExternalInput")
    out = nc.dram_tensor("out", [BS], mybir.dt.float32, kind="ExternalOutput")

    # shard row (p*RPP + t*R + r) -> partition p, tile t, free slot (r, c)
    xv = x.rearrange("(p t r) c -> t p (r c)", p=P, t=NT, r=R)
    ov = out.rearrange("(p f) -> p f", p=P)

    with tile.TileContext(nc) as tc:
        with (
            tc.tile_pool(name="const", bufs=1) as cpool,
            tc.tile_pool(name="xs", bufs=5) as xs,
            tc.tile_pool(name="ys", bufs=3) as ys,
            tc.tile_pool(name="h1", bufs=2) as h1p,
            tc.tile_pool(name="h2", bufs=2) as h2p,
            tc.tile_pool(name="h3", bufs=2) as h3p,
            tc.psum_pool(name="scr", bufs=2) as scr,
            tc.tile_pool(name="res", bufs=1) as res,
        ):
            # tiny w first in the SP HWDGE FIFO (~0.4 us ahead of x tile 0);
            # doubling-replicate on DVE overlaps x tile 0's DMA
            w_t = cpool.tile([P, C], mybir.dt.bfloat16)
            nc.sync.dma_start(out=w_t[:], in_=w[:, :])
            wb_t = cpool.tile([P, F], mybir.dt.bfloat16)
            nc.vector.tensor_copy(out=wb_t[:, 0:C], in_=w_t[:])
            rep = C
            while rep < F:
                n = min(rep, F - rep)
                nc.vector.tensor_copy(
                    out=wb_t[:, rep:rep + n], in_=wb_t[:, 0:n])
                rep += n
            o_t = res.tile([P, RPP], mybir.dt.float32)

            for t in range(NT):
                # endgame: last two tiles shift ACT rows onto DVE
                s_dve = S_DVE if t < NT - 2 else (11 if t == NT - 2 else R)

                x_t = xs.tile([P, F], mybir.dt.bfloat16)
                nc.sync.dma_start(out=x_t[:], in_=xv[t])

                # DVE: bf16 multiply, 2x mode. Tile 0's ACT rows go into
                # their own tile, multiplied first - tile deps are whole-
                # tile, so this starts the ACT chain earlier.
                ya_t = None
                y_t = ys.tile([P, F], mybir.dt.bfloat16, tag="y")
                if t == 0:
                    ya_t = ys.tile([P, (R - s_dve) * C], mybir.dt.bfloat16,
                                   tag="y0a")
                    nc.vector.tensor_mul(
                        ya_t[:], x_t[:, s_dve * C:], wb_t[:, s_dve * C:])
                    nc.vector.tensor_mul(
                        y_t[:, 0:s_dve * C], x_t[:, 0:s_dve * C],
                        wb_t[:, 0:s_dve * C])
                else:
                    nc.vector.tensor_mul(y_t[:], x_t[:], wb_t[:])

                # DVE: three halving levels (2x) for the first s_dve rows
                y3 = y_t[:, 0:s_dve * C].rearrange("p (r c) -> p r c", c=C)
                h1_t = h1p.tile([P, R * H1], mybir.dt.bfloat16)
                h1v = h1_t[:, 0:s_dve * H1].rearrange("p (r c) -> p r c", c=H1)
                nc.vector.tensor_add(h1v, y3[:, :, 0:H1], y3[:, :, H1:C])
                h2_t = h2p.tile([P, R * H2], mybir.dt.bfloat16)
                h2v = h2_t[:, 0:s_dve * H2].rearrange("p (r c) -> p r c", c=H2)
                nc.vector.tensor_add(h2v, h1v[:, :, 0:H2], h1v[:, :, H2:H1])
                h3_t = h3p.tile([P, R * H3], mybir.dt.bfloat16)
                h3v = h3_t[:, 0:s_dve * H3].rearrange("p (r c) -> p r c", c=H3)
                nc.vector.tensor_add(h3v, h2v[:, :, 0:H3], h2v[:, :, H3:H2])

                # DVE: segmented add-reduce of the 64-wide remainders
                nc.vector.tensor_reduce(
                    out=o_t[:, t * R: t * R + s_dve],
                    in_=h3v,
                    axis=mybir.AxisListType.X,
                    op=mybir.AluOpType.add,
                )

                # ACT: accumulate the remaining rows (one 512-sum per row)
                for r in range(R - s_dve):
                    s_t = scr.tile([P, C], mybir.dt.float32, tag="act_s")
                    col = t * R + s_dve + r
                    act_src = (
                        ya_t[:, r * C:(r + 1) * C] if t == 0
                        else y_t[:, (s_dve + r) * C:(s_dve + r + 1) * C]
                    )
                    nc.scalar.activation(
                        out=s_t[:],
                        in_=act_src,
                        func=mybir.ActivationFunctionType.Copy,
                        accum_out=o_t[:, col: col + 1],
                    )
            nc.sync.dma_start(out=ov, in_=o_t[:])
    nc.finalize()
    return nc


def kernel(x: np.ndarray, weight: np.ndarray) -> np.ndarray:
    global _NC_CACHE, LAST_RESULT
    x = np.asarray(x)
    weight = np.asarray(weight, dtype=np.float32)

    x16 = np.ascontiguousarray(x.astype(ml_dtypes.bfloat16))
    w_eff = (C * weight.sum(axis=0)).astype(ml_dtypes.bfloat16)  # [C]
    w_rep = np.ascontiguousarray(np.tile(w_eff, (P, 1)))         # [P, C]

    if _NC_CACHE is None:
        _NC_CACHE = _build()

    in_maps = [
        {"x": x16[i * BS:(i + 1) * BS], "w": w_rep} for i in range(N_CORES)
    ]
    LAST_RESULT = run_bass_kernel_spmd(
        _NC_CACHE, in_maps, core_ids=list(range(N_CORES))
    )
    return np.concatenate([r["out"] for r in LAST_RESULT.results])


# revision 23
# speedup vs baseline: 1.5209x; 1.0114x over previous
"""Trainium2 Bass kernel for nn_Conv2DLayer_16011638080159.

Math: out = C * (x @ weight.sum(0))   with x [524288, 512], weight [9, 512].
Equivalent to a row-wise dot product of x with w_eff = C * weight.sum(0).

Strategy (pure data parallel, per sharding hint):
  - Shard x along the batch axis across 8 NeuronCores (65536 rows each).
  - Host-side prep: fold the tiny K=9 weight sum and the C scale into a
    single [C] vector; cast x and the folded weight to bf16 on the host
    so the device streams half the bytes (~67 MB/core) and DVE's 2x bf16
    mode applies. fp32 accumulation keeps l2 error ~3e-3, inside the
    2e-2 gate.
  - Per core: stream x in [128, 8192] bf16 tiles on the SP HWDGE queue
    ONLY (a single queue streams at ~400 GB/s; splitting across two
    HWDGE queues measured ~25% slower). The tiny [128, 512] weight loads
    first on the same queue and is replicated to [128, 8192] on device.
  - Compute is the bottleneck (~7.2 us/tile, both engines ~100% busy):
      * DVE: bf16 tile-wide multiply (2x), then for the first S_DVE row
        slots a three-level pairwise-halving tree of bf16 adds (also 2x)
        followed by one segmented add-reduce (fp32 accum) on the 64-wide
        remainders.
      * ACT: per-row ACTIVATE(Copy, accum_out) for the other rows
        (~0.8 us/row fixed rate), scratch output in PSUM.
      * Tile 0's ACT rows are multiplied into their own tile first (tile
        deps are whole-tile, so this starts the ACT chain ~4 us earlier);
        the last two tiles shift ACT rows onto DVE, which otherwise
        idles ~12 us while ACT drains its fixed-rate queue.
  - Row mapping: shard row (p*512 + t*R + r) sits at partition p, tile t,
    slot r, so the per-core result tile [128, 512] is exactly the row-major
    view of the per-core output [65536]; one contiguous DMA writes it out.
"""

import numpy as np
import ml_dtypes

import concourse.bacc as bacc
import concourse.bass as bass
import concourse.tile as tile
from concourse import mybir
from concourse.bass_utils import run_bass_kernel_spmd

B = 524288        # total rows
C = 512           # row length
N_CORES = 8
BS = B // N_CORES  # 65536 rows per core
P = 128            # SBUF partitions
RPP = BS // P      # 512 rows per partition
R = 16             # rows per partition per tile
F = R * C          # free elems per tile (2 MB bf16)
NT = RPP // R      # 32 tiles per core
S_DVE = 7          # row slots reduced on DVE in steady state; rest on ACT
H1 = C // 2        # 256
H2 = C // 4        # 128
H3 = C // 8        # 64

_NC_CACHE = None
LAST_RESULT = None  # BassKernelResults of the most recent run (for profiling)


def _build() -> bass.Bass:
    # Bacc (not raw Bass): its compile() pass splits multi-sem waits into
    # EventSemaphore instructions -- the TRN2 ISA allows only 1 wait/inst.
    nc = bacc.Bacc(None, target_bir_lowering=False, debug=False)
    x = nc.dram_tensor("x", [BS, C], mybir.dt.bfloat16, kind="ExternalInput")
    w = nc.dram_tensor("w", [P, C], mybir.dt.bfloat16, kind="ExternalInput")
    out = nc.dram_tensor("out", [BS], mybir.dt.float32, kind="ExternalOutput")

    # shard row (p*RPP + t*R + r) -> partition p, tile t, free slot (r, c)
    xv = x.rearrange("(p t r) c -> t p (r c)", p=P, t=NT, r=R)
    # the same rows at 8-row half-tile granularity for the head
    xh = x.rearrange("(p s r) c -> s p (r c)", p=P, s=RPP // 8, r=8)
    ov = out.rearrange("(p f) -> p f", p=P)

    with tile.TileContext(nc) as tc:
        with (
            tc.tile_pool(name="const", bufs=1) as cpool,
            tc.tile_pool(name="xs", bufs=5) as xs,
            tc.tile_pool(name="ys", bufs=3) as ys,
            tc.tile_pool(name="h1", bufs=2) as h1p,
            tc.tile_pool(name="h2", bufs=2) as h2p,
            tc.tile_pool(name="h3", bufs=2) as h3p,
            tc.psum_pool(name="scr", bufs=2) as scr,
            tc.tile_pool(name="res", bufs=1) as res,
        ):
            # tiny w first in the SP HWDGE FIFO (~0.4 us ahead of x tile 0);
            # doubling-replicate on DVE overlaps x tile 0's DMA
            w_t = cpool.tile([P, C], mybir.dt.bfloat16)
            nc.sync.dma_start(out=w_t[:], in_=w[:, :])
            wb_t = cpool.tile([P, F], mybir.dt.bfloat16)
            nc.vector.tensor_copy(out=wb_t[:, 0:C], in_=w_t[:])
            rep = C
            while rep < F:
                n = min(rep, F - rep)
                nc.vector.tensor_copy(
                    out=wb_t[:, rep:rep + n], in_=wb_t[:, 0:n])
                rep += n
            o_t = res.tile([P, RPP], mybir.dt.float32)

            # head: rows 0..15 as two 8-row half-tiles. The 1 MB DMAs land
            # ~4 us earlier than a full tile would, and each half's ACT
            # rows (3..7) are multiplied into their own small tile first -
            # tile deps are whole-tile, so ACT spins up at ~13.5 us
            # instead of ~18.8 us.
            SH = 3  # DVE row slots per half-tile
            for hs in range(2):
                xh_t = xs.tile([P, F], mybir.dt.bfloat16, tag="x")
                nc.sync.dma_start(out=xh_t[:, 0:8 * C], in_=xh[hs])
                yah_t = ys.tile([P, (8 - SH) * C], mybir.dt.bfloat16,
                                tag="y0a")
                nc.vector.tensor_mul(
                    yah_t[:], xh_t[:, SH * C:8 * C], wb_t[:, SH * C:8 * C])
                ybh_t = ys.tile([P, F], mybir.dt.bfloat16, tag="y")
                nc.vector.tensor_mul(
                    ybh_t[:, 0:SH * C], xh_t[:, 0:SH * C], wb_t[:, 0:SH * C])

                yh3 = ybh_t[:, 0:SH * C].rearrange("p (r c) -> p r c", c=C)
                h1_t = h1p.tile([P, R * H1], mybir.dt.bfloat16)
                h1v = h1_t[:, 0:SH * H1].rearrange("p (r c) -> p r c", c=H1)
                nc.vector.tensor_add(h1v, yh3[:, :, 0:H1], yh3[:, :, H1:C])
                h2_t = h2p.tile([P, R * H2], mybir.dt.bfloat16)
                h2v = h2_t[:, 0:SH * H2].rearrange("p (r c) -> p r c", c=H2)
                nc.vector.tensor_add(h2v, h1v[:, :, 0:H2], h1v[:, :, H2:H1])
                h3_t = h3p.tile([P, R * H3], mybir.dt.bfloat16)
                h3v = h3_t[:, 0:SH * H3].rearrange("p (r c) -> p r c", c=H3)
                nc.vector.tensor_add(h3v, h2v[:, :, 0:H3], h2v[:, :, H3:H2])
                nc.vector.tensor_reduce(
                    out=o_t[:, hs * 8: hs * 8 + SH],
                    in_=h3v,
                    axis=mybir.AxisListType.X,
                    op=mybir.AluOpType.add,
                )
                for r in range(8 - SH):
                    s_t = scr.tile([P, C], mybir.dt.float32, tag="act_s")
                    col = hs * 8 + SH + r
                    nc.scalar.activation(
                        out=s_t[:],
                        in_=yah_t[:, r * C:(r + 1) * C],
                        func=mybir.ActivationFunctionType.Copy,
                        accum_out=o_t[:, col: col + 1],
                    )

            for t in range(1, NT):
                # endgame: last two tiles shift ACT rows onto DVE
                s_dve = S_DVE if t < NT - 2 else (11 if t == NT - 2 else 15)

                x_t = xs.tile([P, F], mybir.dt.bfloat16, tag="x")
                nc.sync.dma_start(out=x_t[:], in_=xv[t])

                # DVE: bf16 multiply, 2x mode
                y_t = ys.tile([P, F], mybir.dt.bfloat16, tag="y")
                nc.vector.tensor_mul(y_t[:], x_t[:], wb_t[:])

                # DVE: three halving levels (2x) for the first s_dve rows
                y3 = y_t[:, 0:s_dve * C].rearrange("p (r c) -> p r c", c=C)
                h1_t = h1p.tile([P, R * H1], mybir.dt.bfloat16)
                h1v = h1_t[:, 0:s_dve * H1].rearrange("p (r c) -> p r c", c=H1)
                nc.vector.tensor_add(h1v, y3[:, :, 0:H1], y3[:, :, H1:C])
                h2_t = h2p.tile([P, R * H2], mybir.dt.bfloat16)
                h2v = h2_t[:, 0:s_dve * H2].rearrange("p (r c) -> p r c", c=H2)
                nc.vector.tensor_add(h2v, h1v[:, :, 0:H2], h1v[:, :, H2:H1])
                h3_t = h3p.tile([P, R * H3], mybir.dt.bfloat16)
                h3v = h3_t[:, 0:s_dve * H3].rearrange("p (r c) -> p r c", c=H3)
                nc.vector.tensor_add(h3v, h2v[:, :, 0:H3], h2v[:, :, H3:H2])

                # DVE: segmented add-reduce of the 64-wide remainders
                nc.vector.tensor_reduce(
                    out=o_t[:, t * R: t * R + s_dve],
                    in_=h3v,
                    axis=mybir.AxisListType.X,
                    op=mybir.AluOpType.add,
                )

                # ACT: accumulate the remaining rows (one 512-sum per row)
                for r in range(R - s_dve):
                    s_t = scr.tile([P, C], mybir.dt.float32, tag="act_s")
                    col = t * R + s_dve + r
                    nc.scalar.activation(
                        out=s_t[:],
                        in_=y_t[:, (s_dve + r) * C:(s_dve + r + 1) * C],
                        func=mybir.ActivationFunctionType.Copy,
                        accum_out=o_t[:, col: col + 1],
                    )
            nc.sync.dma_start(out=ov, in_=o_t[:])
    nc.finalize()
    return nc


def kernel(x: np.ndarray, weight: np.ndarray) -> np.ndarray:
    global _NC_CACHE, LAST_RESULT
    x = np.asarray(x)
    weight = np.asarray(weight, dtype=np.float32)

    x16 = np.ascontiguousarray(x.astype(ml_dtypes.bfloat16))
    w_eff = (C * weight.sum(axis=0)).astype(ml_dtypes.bfloat16)  # [C]
    w_rep = np.ascontiguousarray(np.tile(w_eff, (P, 1)))         # [P, C]

    if _NC_CACHE is None:
        _NC_CACHE = _build()

    in_maps = [
        {"x": x16[i * BS:(i + 1) * BS], "w": w_rep} for i in range(N_CORES)
    ]
    LAST_RESULT = run_bass_kernel_spmd(
        _NC_CACHE, in_maps, core_ids=list(range(N_CORES))
    )
    return np.concatenate([r["out"] for r in LAST_RESULT.results])
